# revision 15
# baseline (speedup 1.0000x reference)
"""DeepSeekMoE forward on 8 Trainium2 NeuronCores (Bass/Tile).

Strategy (expert-parallel, host dispatch/combine):
  - Router (sigmoid scores + top-4 + gating) computed on host with jax-CPU,
    bitwise-matching the reference's op sequence.
  - 24 uniform "FFN jobs": 16 routed experts (tokens gathered per expert,
    padded to capacity C) + 2 shared experts x 4 token-shards of 2048.
  - Each core runs 3 jobs: 1 shared-expert shard + its 2 routed experts.
    Per job: H^T = relu(W1^T X^T + b1); Y^T = W2^T H^T + b2, computed with
    feature-major fp32r matmuls (full-rate on trn2 PE for moving dim >=256).
  - Host scatters routed outputs back with gating weights and adds the
    residual + shared outputs.
"""

import numpy as np

D_MODEL, D_FF, NS, NR, KR = 2048, 1408, 2, 16, 4
P = 128
DT = D_MODEL // P  # 16
FT = D_FF // P     # 11
NCORES = 8
JOBS = 3           # per core: [shared shard, routed expert A, routed expert B]
SH_TOK = 2048      # shared-expert shard size (per core)

_prog_cache = {}
LAST_RESULT = None  # BassKernelResults of the most recent device run


def _ensure_ntff_hook():
    """This image's `antenv` lacks the `axon_hooks` get/set registry that
    `run_bass_kernel_spmd(trace=True)` imports under axon; install an
    equivalent shim backed by the libaxon ctypes profiler so tracing works
    (and BASS_TRACE=1 doesn't crash the run)."""
    try:
        from antenv.axon_hooks import get_axon_ntff_profile_hook  # noqa: F401
        return
    except ImportError:
        pass
    import sys
    import types
    try:
        import antenv
        mod = types.ModuleType("antenv.axon_hooks")
        _hook = [None]
        mod.set_axon_ntff_profile_hook = lambda h: _hook.__setitem__(0, h)
        mod.get_axon_ntff_profile_hook = lambda: _hook[0]
        sys.modules["antenv.axon_hooks"] = mod
        antenv.axon_hooks = mod
        from trn_agent_boot.trn_boot import _ntff_profile_via_ctypes
        mod.set_axon_ntff_profile_hook(
            _ntff_profile_via_ctypes("/opt/axon/libaxon_pjrt.so")
        )
    except Exception:
        pass


def _plan_chunks(block):
    """Split a block (multiple of 128, >=256) into moving-dim chunks in
    {256, 384, 512} so every fp32r matmul has moving dim >= 256."""
    n8 = block // P
    assert block % P == 0 and n8 >= 2
    out = []
    while n8 > 0:
        if n8 in (2, 3, 4):
            out.append(n8 * P)
            n8 = 0
        elif n8 == 5:
            out += [2 * P, 3 * P]
            n8 = 0
        else:
            out.append(4 * P)
            n8 -= 4
    return out


def _plan_blocks(C):
    """Split capacity C into token blocks of at most 1152 (SBUF budget),
    each a multiple of 128 and >= 256."""
    blocks = []
    rem = C
    while rem > 0:
        if rem <= 1152:
            blocks.append(rem)
            rem = 0
        elif rem - 1024 >= 256:
            blocks.append(1024)
            rem -= 1024
        else:
            b = (rem // 2 // P) * P
            blocks += [b, rem - b]
            rem = 0
    assert sum(blocks) == C and all(b >= 256 and b % P == 0 for b in blocks)
    return blocks


def _build_program(C):
    import concourse.mybir as mybir
    import concourse.tile as tile
    from concourse import bacc

    F32 = mybir.dt.float32
    F32R = mybir.dt.float32r
    Relu = mybir.ActivationFunctionType.Relu
    Identity = mybir.ActivationFunctionType.Identity

    job_tokens = [SH_TOK, C, C]
    job_blocks = [_plan_blocks(t) for t in job_tokens]

    nc = bacc.Bacc(None, target_bir_lowering=False)
    xt = nc.dram_tensor("xt", [JOBS, P, DT, C], F32R, kind="ExternalInput")
    w1 = nc.dram_tensor("w1", [JOBS, FT, P, DT, P], F32R, kind="ExternalInput")
    w2 = nc.dram_tensor("w2", [JOBS, DT, P, FT, P], F32R, kind="ExternalInput")
    b1 = nc.dram_tensor("b1", [P, JOBS * FT], F32, kind="ExternalInput")
    b2 = nc.dram_tensor("b2", [P, JOBS * DT], F32, kind="ExternalInput")
    yt = nc.dram_tensor("yt", [JOBS, DT, P, C], F32, kind="ExternalOutput")

    with tile.TileContext(nc) as tc:
        with (
            tc.tile_pool(name="const", bufs=1) as const,
            tc.tile_pool(name="x", bufs=1) as xpool,
            tc.tile_pool(name="h", bufs=1) as hpool,
            tc.tile_pool(name="w1p", bufs=3) as w1pool,
            tc.tile_pool(name="w2p", bufs=4) as w2pool,
            tc.tile_pool(name="y", bufs=3) as ypool,
            tc.tile_pool(name="ps", bufs=6, space="PSUM") as pspool,
        ):
            b1t = const.tile([P, JOBS * FT], F32)
            nc.gpsimd.dma_start(b1t[:], b1[:, :])
            b2t = const.tile([P, JOBS * DT], F32)
            nc.gpsimd.dma_start(b2t[:], b2[:, :])

            first_block = True
            for j in range(JOBS):
                off = 0
                for blk in job_blocks[j]:
                    chunks = _plan_chunks(blk)
                    # first f-tile's weights before the X block so the first
                    # matmul group can start as soon as x slices land
                    w1_first = w1pool.tile([P, DT, P], F32R, tag="w1")
                    nc.sync.dma_start(w1_first[:], w1[j, 0])
                    # X rides the GPSIMD SWDGE ring: the SP ring's sequencer
                    # stalls on each pool-WAR-gated weight DMA, so anything
                    # queued behind the weights only transfers at consumption
                    # pace. On its own ring the next block's X lands during
                    # the previous block's mm2 phase.
                    xt_t = xpool.tile([P, DT, blk], F32R, tag="x")
                    if first_block:
                        # per-ko sub-DMAs: finer completion granularity so the
                        # first matmul group isn't gated on the whole 8MB block
                        for ko in range(DT):
                            nc.gpsimd.dma_start(
                                xt_t[:, ko], xt[j, :, ko, off : off + blk]
                            )
                        first_block = False
                    else:
                        nc.gpsimd.dma_start(
                            xt_t[:], xt[j, :, :, off : off + blk]
                        )
                    xts = [xt_t[:, ko] for ko in range(DT)]
                    h_t = hpool.tile([P, FT, blk], F32R, tag="h")

                    for ft in range(FT):
                        if ft == 0:
                            w1_t = w1_first
                        else:
                            w1_t = w1pool.tile([P, DT, P], F32R, tag="w1")
                            nc.sync.dma_start(w1_t[:], w1[j, ft])
                        coff = 0
                        for ch in chunks:
                            ps = pspool.tile([P, 512], F32, tag="ps")
                            for ko in range(DT):
                                nc.tensor.matmul(
                                    ps[:, :ch],
                                    w1_t[:, ko],
                                    xts[ko][:, coff : coff + ch],
                                    start=(ko == 0),
                                    stop=(ko == DT - 1),
                                )
                            nc.scalar.activation(
                                h_t[:, ft, coff : coff + ch],
                                ps[:, :ch],
                                Relu,
                                bias=b1t[:, j * FT + ft : j * FT + ft + 1],
                            )
                            coff += ch

                    for dtile in range(DT):
                        w2_t = w2pool.tile([P, FT, P], F32R, tag="w2")
                        nc.sync.dma_start(w2_t[:], w2[j, dtile])
                        y_t = ypool.tile([P, 1152], F32, tag="y")
                        coff = 0
                        for ch in chunks:
                            ps = pspool.tile([P, 512], F32, tag="ps")
                            for ko in range(FT):
                                nc.tensor.matmul(
                                    ps[:, :ch],
                                    w2_t[:, ko],
                                    h_t[:, ko, coff : coff + ch],
                                    start=(ko == 0),
                                    stop=(ko == FT - 1),
                                )
                            nc.scalar.activation(
                                y_t[:, coff : coff + ch],
                                ps[:, :ch],
                                Identity,
                                bias=b2t[:, j * DT + dtile : j * DT + dtile + 1],
                            )
                            coff += ch
                        # Y rides the ACT HW-DGE ring: keeps the SP ring free
                        # so the next block's X transfer isn't stuck behind
                        # sem-gated output writes (v3 showed 10-16us PE gaps
                        # at every block boundary from that FIFO blockage)
                        nc.scalar.dma_start(
                            yt[j, dtile, :, off : off + blk], y_t[:, :blk]
                        )
                    off += blk

    nc.compile()
    return nc


def _routing(flat, centroids, bias):
    """Replicate the reference router bitwise: jax-CPU sigmoid scores,
    stable top-4 (argsort matches lax.top_k tie-breaking), normalized gates."""
    import jax
    import jax.numpy as jnp

    cpu = jax.devices("cpu")[0]
    with jax.default_device(cpu):
        scores = np.asarray(
            jax.nn.sigmoid(jnp.asarray(flat) @ jnp.asarray(centroids).T)
            + jnp.asarray(bias)
        )
    idx = np.argsort(-scores, axis=-1, kind="stable")[:, :KR]
    vals = np.take_along_axis(scores, idx, axis=-1)
    gating = vals / np.maximum(vals.sum(-1, keepdims=True, dtype=np.float32), 1e-8)
    return idx.astype(np.int32), gating.astype(np.float32)


def _feat_major(x_td):
    """[T, D] (rows=tokens) -> [P, D//P, T] feature-major device layout."""
    d = x_td.shape[1]
    return np.ascontiguousarray(x_td.T.reshape(d // P, P, -1).transpose(1, 0, 2))


def _w_tiles(w, kdim, mdim):
    """[K, M] -> [M//P, P(k_inner), K//P, P(m_inner)] lhsT tile layout."""
    kt, mt = kdim // P, mdim // P
    return np.ascontiguousarray(
        w.reshape(kt, P, mt, P).transpose(2, 1, 0, 3)
    )


def kernel(u, shared_w1, shared_b1, shared_w2, shared_b2,
           routed_w1, routed_b1, routed_w2, routed_b2, centroids, bias):
    from concourse.bass_utils import run_bass_kernel_spmd

    _ensure_ntff_hook()
    u = np.asarray(u, dtype=np.float32)
    b, s, d = u.shape
    flat = u.reshape(-1, d)
    T = flat.shape[0]

    idx, gating = _routing(flat, np.asarray(centroids, np.float32),
                           np.asarray(bias, np.float32))

    # per-expert token lists (ascending token id) and their gate values
    tok_lists, gate_lists = [], []
    for e in range(NR):
        hit = idx == e                        # [T, KR]
        rows = np.nonzero(hit.any(axis=1))[0]
        g = gating[hit].reshape(-1)           # row-major -> ascending token id
        tok_lists.append(rows)
        gate_lists.append(g.astype(np.float32))

    max_count = max(len(r) for r in tok_lists)
    C = max(256, -(-max_count // P) * P)
    key = (C,)
    if key not in _prog_cache:
        _prog_cache[key] = _build_program(C)
    nc = _prog_cache[key]

    sw1 = np.asarray(shared_w1, np.float32)
    sb1 = np.asarray(shared_b1, np.float32)
    sw2 = np.asarray(shared_w2, np.float32)
    sb2 = np.asarray(shared_b2, np.float32)
    rw1 = np.asarray(routed_w1, np.float32)
    rb1 = np.asarray(routed_b1, np.float32)
    rw2 = np.asarray(routed_w2, np.float32)
    rb2 = np.asarray(routed_b2, np.float32)

    rw1_t = [_w_tiles(rw1[e], D_MODEL, D_FF) for e in range(NR)]
    rw2_t = [_w_tiles(rw2[e], D_FF, D_MODEL) for e in range(NR)]
    sw1_t = [_w_tiles(sw1[n], D_MODEL, D_FF) for n in range(NS)]
    sw2_t = [_w_tiles(sw2[n], D_FF, D_MODEL) for n in range(NS)]

    in_maps = []
    for core in range(NCORES):
        sh_e = core % NS
        sh_off = (core // NS) * SH_TOK
        e0, e1 = 2 * core, 2 * core + 1

        xt = np.zeros((JOBS, P, DT, C), np.float32)
        xt[0, :, :, :SH_TOK] = _feat_major(flat[sh_off : sh_off + SH_TOK])
        for jslot, e in ((1, e0), (2, e1)):
            rows = tok_lists[e]
            if len(rows):
                xt[jslot, :, :, : len(rows)] = _feat_major(flat[rows])

        w1 = np.stack([sw1_t[sh_e], rw1_t[e0], rw1_t[e1]])
        w2 = np.stack([sw2_t[sh_e], rw2_t[e0], rw2_t[e1]])
        b1m = np.stack([sb1[sh_e], rb1[e0], rb1[e1]])   # [3, 1408]
        b2m = np.stack([sb2[sh_e], rb2[e0], rb2[e1]])   # [3, 2048]
        b1m = np.ascontiguousarray(b1m.reshape(JOBS * FT, P).T)  # [P, 33]
        b2m = np.ascontiguousarray(b2m.reshape(JOBS * DT, P).T)  # [P, 48]

        in_maps.append({"xt": xt, "w1": w1, "w2": w2, "b1": b1m, "b2": b2m})

    res = run_bass_kernel_spmd(nc, in_maps, core_ids=list(range(NCORES)))
    global LAST_RESULT
    LAST_RESULT = res

    out = flat.copy()
    for core in range(NCORES):
        ytc = res.results[core]["yt"]          # [JOBS, DT, P, C]
        sh_off = (core // NS) * SH_TOK
        out[sh_off : sh_off + SH_TOK] += (
            ytc[0].reshape(D_MODEL, C)[:, :SH_TOK].T
        )
        for jslot, e in ((1, 2 * core), (2, 2 * core + 1)):
            rows = tok_lists[e]
            if len(rows):
                ye = ytc[jslot].reshape(D_MODEL, C)[:, : len(rows)].T
                out[rows] += gate_lists[e][:, None] * ye

    return out.reshape(b, s, d)
